# revision 24
# baseline (speedup 1.0000x reference)
"""BLOBLoss Trainium2 kernel (stride-8 subsample, wide-DVE formulation).

Math (mirrors the reference): scores[r] = mean of 3 refine heads, thresholded
at 0.3; M[y,x] = sum_r s_r*[y1<=y<y2]*[x1<=x<x2].  The loss reads M only
through its stride-8 subsample SUB = M[::8,::8] (row/col maxima thresholded
at the normalized 0.5 level) and the global min/max used to normalize.
Min/max over the stride-8 grid instead of the full 1024 grid changes the
final scalar by ~1e-5 relative (tolerance 2e-2), so only the 128x128 SUB is
computed.

Structure (driven by measured TRN2 costs: ~290ns/DVE instruction flat, wide
packed fp16 tensor_tensor at 0.52ns/elem, matmuls ~32ns back-to-back,
~680ns/DMA serialized per queue):
  - inputs arrive as 4 packed DMAs on 4 different engine DGE queues;
  - ROIs sorted by x1 so each 128-ROI ktile's x-windows fit a narrow XW-col
    region: x masks built with 3 narrow wide-ops + a broadcast score mult;
  - y side uses the +-step identity yw = [i>=cy1] + [i<cy2] - 1: only 2 wide
    compares, the window product is absorbed into PE as two accumulating
    matmul terms per ktile plus a rank-1 ones-correction (-1 x colsum(xws));
  - SUB accumulates in one PSUM bank; min/max/threshold tail uses a PE
    transpose (identity matmul) for the column maxima.
Per-core: one valid channel (VCP=ceil(nv/8)); invalid-channel blob log terms
round-robined; each core emits one partial scalar, host sums the 8.
"""

import math
import sys

import numpy as np

for _p in ("/opt/trn_rl_repo",):
    if _p not in sys.path:
        sys.path.append(_p)

EPS = 1e-6
NCORES = 8

_PROG_CACHE = {}


def _build_program(VCP, NIP, NKT, C, XW, S8):
    import concourse.bacc as bacc
    import concourse.bass as bass
    import concourse.bass_isa as bass_isa
    import concourse.mybir as mybir
    from concourse import tile

    dt = mybir.dt
    f32, f16 = dt.float32, dt.float16
    AF = mybir.ActivationFunctionType
    Op = mybir.AluOpType
    Ax = mybir.AxisListType

    NX = NKT * XW
    NY = NKT * 128
    NB = (2 * VCP + 2 * NIP) * 128

    nc = bacc.Bacc("TRN2", target_bir_lowering=False, debug=False,
                   num_devices=NCORES)

    def din(name, shape, dtp=f32):
        return nc.dram_tensor(name, shape, dtp, kind="ExternalInput").ap()

    # coords pack: x1l|x2l|y1|y2 columns + local x iota + 128-iota + ident
    coords_d = din("coords", [128, 4 * NKT + XW + 128 + 128], f16)
    y1r_d = din("y1r", [128, NY], f16)
    y2r_d = din("y2r", [128, NY], f16)
    refine_d = din("refine", [128, NKT * 3 * VCP], f16)
    labels_d = din("labels", [1, C])
    packb_d = din("packb", [128, NB])  # blobp|blobpT|blobn|blobnT
    out_d = nc.dram_tensor("out", [1, 1], f32, kind="ExternalOutput").ap()

    with tile.TileContext(nc) as tc:
        with (
            tc.tile_pool(name="const", bufs=1) as cp,
            tc.tile_pool(name="work", bufs=4) as wp,
            tc.tile_pool(name="psum", bufs=2, space=bass.MemorySpace.PSUM) as pp,
            tc.tile_pool(name="psums", bufs=1, space=bass.MemorySpace.PSUM) as pps,
        ):
            # ---- input DMAs on separate engine queues ----
            coords = cp.tile([128, 4 * NKT + XW + 128 + 128], f16)
            nc.sync.dma_start(coords[:], coords_d)
            refS = cp.tile([128, NKT * 3 * VCP], f16)
            nc.scalar.dma_start(refS[:], refine_d)
            labels = cp.tile([1, C], f32)
            nc.gpsimd.dma_start(labels[:], labels_d)
            y1r_t = cp.tile([128, NY], f16)
            y2r_t = cp.tile([128, NY], f16)
            QY = NY // 4
            for c4 in range(4):
                sq = slice(c4 * QY, (c4 + 1) * QY)
                nc.sync.dma_start(y1r_t[:, sq], y1r_d[:, sq])
                nc.scalar.dma_start(y2r_t[:, sq], y2r_d[:, sq])
            packb = cp.tile([128, NB], f32)
            nc.gpsimd.dma_start(packb[:], packb_d)

            x1l = coords[:, 0 * NKT:1 * NKT]
            x2l = coords[:, 1 * NKT:2 * NKT]
            y1c = coords[:, 2 * NKT:3 * NKT]
            y2c = coords[:, 3 * NKT:4 * NKT]
            ioxl = coords[:, 4 * NKT:4 * NKT + XW]
            io128 = coords[:, 4 * NKT + XW:4 * NKT + XW + 128]
            ident = coords[:, 4 * NKT + XW + 128:4 * NKT + XW + 256]
            iox_b = ioxl.unsqueeze(1).to_broadcast([128, NKT, XW])
            x1r = x1l.unsqueeze(2).to_broadcast([128, NKT, XW])
            x2r = x2l.unsqueeze(2).to_broadcast([128, NKT, XW])
            blobp = packb[:, 0:VCP * 128].rearrange(
                "p (v w) -> p v w", v=VCP)
            blobpT = packb[:, VCP * 128:2 * VCP * 128].rearrange(
                "p (v w) -> p v w", v=VCP)
            blobn = packb[:, 2 * VCP * 128:(2 * VCP + NIP) * 128].rearrange(
                "p (v w) -> p v w", v=NIP)
            blobnT = packb[:, (2 * VCP + NIP) * 128:NB].rearrange(
                "p (v w) -> p v w", v=NIP)

            ones_r = cp.tile([1, 128], f32)
            nc.vector.memset(ones_r[:], 1.0)
            mones_r = cp.tile([1, 128], f32)
            nc.vector.memset(mones_r[:], -1.0)
            ones_c32 = cp.tile([128, 1], f32)
            nc.vector.memset(ones_c32[:], 1.0)
            ones_c16 = cp.tile([128, 1], f16)
            nc.vector.memset(ones_c16[:], 1.0)

            # ---- x-side scoreless masks (only need coords; start ASAP) ----
            gx = wp.tile([128, NKT, XW], f16, tag="gx")
            nc.vector.tensor_tensor(gx[:], iox_b, x1r, op=Op.is_ge)
            ux = wp.tile([128, NKT, XW], f16, tag="ux")
            nc.vector.tensor_tensor(ux[:], iox_b, x2r, op=Op.is_lt)
            xm = wp.tile([128, NKT, XW], f16, tag="xm")
            nc.vector.tensor_tensor(xm[:], gx[:], ux[:], op=Op.mult)

            # ---- scores: (sum of 3 heads >= 0.9) * sum/3 -> fp16 ----
            ref4 = refS[:].rearrange("p (k h v) -> p k h v", k=NKT, h=3)
            ssum = wp.tile([128, NKT * VCP], f32, tag="ssum")
            ssum3 = ssum[:].rearrange("p (k v) -> p k v", k=NKT)
            nc.vector.tensor_add(ssum3, ref4[:, :, 0, :], ref4[:, :, 1, :])
            nc.vector.tensor_add(ssum3, ssum3, ref4[:, :, 2, :])
            msk = wp.tile([128, NKT * VCP], f32, tag="msk")
            nc.vector.tensor_scalar(msk[:], ssum[:], 0.9, 1.0 / 3.0,
                                    op0=Op.is_ge, op1=Op.mult)
            sc16 = cp.tile([128, NKT * VCP], f16)
            nc.vector.tensor_mul(sc16[:], ssum[:], msk[:])

            mxl = cp.tile([128, VCP], f32)
            myl = cp.tile([128, VCP], f32)

            for v in range(VCP):
                # ---- x-side score multiply ----
                xws = cp.tile([128, NKT, XW], f16, tag=f"xws{v}",
                              name=f"xws{v}")
                scb = sc16[:].rearrange("p (k v) -> p k v", k=NKT)[
                    :, :, v:v + 1].to_broadcast([128, NKT, XW])
                nc.vector.tensor_tensor(xws[:], xm[:], scb, op=Op.mult)

                # rank-1 correction pieces: R1[j] = sum_r xws[r, j]
                pssub = pp.tile([128, 128], f32, tag="sub")
                nc.vector.memset(pssub[:], 0.0)
                psr1 = pps.tile([1, 128], f32, tag="r1")
                nc.vector.memset(psr1[:], 0.0)
                for kt in range(NKT):
                    nc.tensor.matmul(psr1[:, S8[kt]:S8[kt] + XW],
                                     ones_c16[:], xws[:, kt, :],
                                     start=False, stop=(kt == NKT - 1),
                                     skip_group_check=True)
                r1sb = wp.tile([1, 128], f32, tag="r1sb")
                nc.vector.tensor_copy(r1sb[:], psr1[:])
                nc.tensor.matmul(pssub[:], mones_r[:], r1sb[:],
                                 start=False, stop=False,
                                 skip_group_check=True)

                # ---- y-side +-step masks, 2 chunks; matmuls per ktile ----
                KH = NKT // 4
                for c2 in range(4):
                    sl = slice(c2 * KH * 128, (c2 + 1) * KH * 128)
                    io_b = io128.unsqueeze(1).to_broadcast([128, KH, 128])
                    y1v = y1r_t[:, sl].rearrange("p (k x) -> p k x", k=KH)
                    y2v = y2r_t[:, sl].rearrange("p (k x) -> p k x", k=KH)
                    gy = wp.tile([128, KH, 128], f16, tag="gy",
                                 name=f"gy{v}_{c2}")
                    nc.vector.tensor_tensor(gy[:], io_b, y1v, op=Op.is_ge)
                    uy = wp.tile([128, KH, 128], f16, tag="uy",
                                 name=f"uy{v}_{c2}")
                    nc.vector.tensor_tensor(uy[:], io_b, y2v, op=Op.is_lt)
                    for k2 in range(KH):
                        kt = c2 * KH + k2
                        nc.tensor.matmul(pssub[:, S8[kt]:S8[kt] + XW],
                                         gy[:, k2, :], xws[:, kt, :],
                                         start=False, stop=False,
                                         skip_group_check=True)
                        nc.tensor.matmul(pssub[:, S8[kt]:S8[kt] + XW],
                                         uy[:, k2, :], xws[:, kt, :],
                                         start=False,
                                         stop=(kt == NKT - 1),
                                         skip_group_check=True)

                # ---- min/max, threshold, row/col masks ----
                rn16 = wp.tile([128, 128], f16, tag="rn16")
                nc.vector.tensor_copy(rn16[:], pssub[:])
                psT = pp.tile([128, 128], f16, tag="pst")
                nc.tensor.transpose(psT[:], rn16[:], ident)
                colMax = wp.tile([128, 1], f32, tag="colMax")
                nc.vector.tensor_reduce(colMax[:], pssub[:], axis=Ax.X,
                                        op=Op.max)
                colMin = wp.tile([128, 1], f32, tag="colMin")
                nc.vector.tensor_reduce(colMin[:], pssub[:], axis=Ax.X,
                                        op=Op.min, negate=True)
                gmax = wp.tile([1, 1], f32, tag="gmax")
                nc.gpsimd.tensor_reduce(gmax[:], colMax[:], axis=Ax.XYZWC,
                                        op=Op.max)
                gmin_neg = wp.tile([1, 1], f32, tag="gmin")
                nc.gpsimd.tensor_reduce(gmin_neg[:], colMin[:], axis=Ax.XYZWC,
                                        op=Op.max)
                # rowmax >= gmin + .5*(gmax-gmin+eps) = .5*(gmax+gmin)+eps/2
                thr = wp.tile([1, 1], f32, tag="thr")
                nc.vector.tensor_sub(thr[:], gmax[:], gmin_neg[:])
                nc.vector.tensor_scalar(thr[:], thr[:], 0.5, EPS / 2,
                                        op0=Op.mult, op1=Op.add)
                pthr = pps.tile([128, 1], f32, tag="small")
                nc.tensor.matmul(pthr[:], ones_r[:], thr[:],
                                 start=True, stop=True)
                thrb = wp.tile([128, 1], f32, tag="thrb")
                nc.vector.tensor_copy(thrb[:], pthr[:])
                nc.vector.tensor_scalar(myl[:, v:v + 1], colMax[:], thrb[:],
                                        None, op0=Op.is_ge)
                redT = wp.tile([128, 1], f32, tag="redT")
                nc.vector.tensor_reduce(redT[:], psT[:], axis=Ax.X,
                                        op=Op.max)
                nc.vector.tensor_scalar(mxl[:, v:v + 1], redT[:], thrb[:],
                                        None, op0=Op.is_ge)

            # ---- divisors from labels (early; fold -1/128 into them) ----
            vmf = wp.tile([1, C], f32, tag="vmf")
            nc.vector.tensor_scalar(vmf[:], labels[:], 1.0, None,
                                    op0=Op.is_equal)
            vc = wp.tile([1, 1], f32, tag="vc")
            nc.vector.tensor_reduce(vc[:], vmf[:], axis=Ax.X, op=Op.add)
            nvc = wp.tile([1, 1], f32, tag="nvc")
            nc.vector.tensor_scalar(nvc[:], vc[:], -1.0, float(C),
                                    op0=Op.mult, op1=Op.add)
            ivs = wp.tile([1, 1], f32, tag="ivs")
            nc.vector.reciprocal(ivs[:], vc[:])
            nc.vector.tensor_scalar_mul(ivs[:], ivs[:], -1.0 / 128.0)
            invs = wp.tile([1, 1], f32, tag="invs")
            nc.vector.reciprocal(invs[:], nvc[:])
            nc.vector.tensor_scalar_mul(invs[:], invs[:], -1.0 / 128.0)

            # ---- blob side: max first, clip after (clip is monotonic) ----
            myb = wp.tile([128, VCP], f32, tag="myb")
            nc.vector.tensor_reduce(myb[:], blobp, axis=Ax.X, op=Op.max)
            mxb = wp.tile([128, VCP], f32, tag="mxb")
            nc.vector.tensor_reduce(mxb[:], blobpT, axis=Ax.X, op=Op.max)
            nc.vector.tensor_scalar(myb[:], myb[:], EPS, 1.0 - EPS,
                                    op0=Op.max, op1=Op.min)
            nc.vector.tensor_scalar(mxb[:], mxb[:], EPS, 1.0 - EPS,
                                    op0=Op.max, op1=Op.min)
            lnx = wp.tile([128, VCP], f32, tag="lnx")
            nc.scalar.activation(lnx[:], mxb[:], AF.Ln)
            lny = wp.tile([128, VCP], f32, tag="lny")
            nc.scalar.activation(lny[:], myb[:], AF.Ln)
            mybn = wp.tile([128, NIP], f32, tag="mybn")
            nc.vector.tensor_reduce(mybn[:], blobn, axis=Ax.X, op=Op.max)
            mxbn = wp.tile([128, NIP], f32, tag="mxbn")
            nc.vector.tensor_reduce(mxbn[:], blobnT, axis=Ax.X, op=Op.max)
            nc.vector.tensor_scalar(mybn[:], mybn[:], EPS, 1.0 - EPS,
                                    op0=Op.max, op1=Op.min)
            nc.vector.tensor_scalar(mxbn[:], mxbn[:], EPS, 1.0 - EPS,
                                    op0=Op.max, op1=Op.min)
            lnxn = wp.tile([128, NIP], f32, tag="lnxn")
            nc.scalar.activation(lnxn[:], mxbn[:], AF.Ln, bias=1.0, scale=-1.0)
            lnyn = wp.tile([128, NIP], f32, tag="lnyn")
            nc.scalar.activation(lnyn[:], mybn[:], AF.Ln, bias=1.0, scale=-1.0)
            nc.vector.tensor_add(lnxn[:], lnxn[:], lnyn[:])
            nv_ps = pps.tile([128, 1], f32, tag="small")
            nc.tensor.matmul(nv_ps[0:NIP, :], lnxn[:], ones_c32[:],
                             start=True, stop=True)
            snv = wp.tile([NIP, 1], f32, tag="snv")
            nc.vector.tensor_copy(snv[:], nv_ps[0:NIP, :])
            Sn = wp.tile([1, 1], f32, tag="Sn")
            nc.gpsimd.tensor_reduce(Sn[:], snv[:], axis=Ax.XYZWC, op=Op.add)

            # ---- final: Sp via PE dot products, combine, store ----
            psd = pps.tile([1, 2 * VCP], f32, tag="small")
            for v in range(VCP):
                nc.tensor.matmul(psd[:, v:v + 1], lnx[:, v:v + 1],
                                 mxl[:, v:v + 1], start=True, stop=True,
                                 skip_group_check=True)
                nc.tensor.matmul(psd[:, VCP + v:VCP + v + 1], lny[:, v:v + 1],
                                 myl[:, v:v + 1], start=True, stop=True,
                                 skip_group_check=True)
            sp2 = wp.tile([1, 2 * VCP], f32, tag="sp2")
            nc.vector.tensor_copy(sp2[:], psd[:])
            Sp = wp.tile([1, 1], f32, tag="Sp")
            nc.vector.tensor_reduce(Sp[:], sp2[:], axis=Ax.X, op=Op.add)
            nc.vector.tensor_mul(Sp[:], Sp[:], ivs[:])
            nc.vector.tensor_mul(Sn[:], Sn[:], invs[:])
            tot = wp.tile([1, 1], f32, tag="tot")
            nc.vector.tensor_add(tot[:], Sp[:], Sn[:])
            nc.sync.dma_start(out_d, tot[:])

    nc.compile()
    return nc


def _get_program(key):
    if key not in _PROG_CACHE:
        VCP, NIP, NKT, C, XW, S8 = key
        _PROG_CACHE[key] = _build_program(VCP, NIP, NKT, C, XW, S8)
    return _PROG_CACHE[key]


def make_in_maps(mil_result, refine_result, blob_conv, rois, labels, H, W):
    """Host-side sharding: slice/relayout full inputs into 8 per-core maps."""
    refine = np.asarray(refine_result, np.float32)
    blob = np.asarray(blob_conv, np.float32)
    rois = np.asarray(rois, np.float32)
    labels = np.asarray(labels)
    K, R, C1 = refine.shape
    C = labels.shape[1]
    assert int(H) == 1024 and int(W) == 1024
    h, w = blob.shape[-2:]
    assert h == 128 and w == 128

    base = 1 if C1 != C else 0
    valid = labels[0] == 1
    vidx = np.nonzero(valid)[0]
    iidx = np.nonzero(~valid)[0]
    nv, ni = len(vidx), len(iidx)
    VCP = max(1, math.ceil(nv / NCORES))
    NIP = max(1, math.ceil(ni / NCORES))
    RP = math.ceil(R / 128) * 128
    NKT = RP // 128

    b = rois[:, 1:5].astype(np.int32)  # int() truncation, like the reference
    cx1 = np.full(RP, 200.0, np.float32)
    cx2 = np.zeros(RP, np.float32)
    cy1 = np.full(RP, 200.0, np.float32)
    cy2 = np.zeros(RP, np.float32)
    cx1[:R] = -(-b[:, 0] // 8)
    cy1[:R] = -(-b[:, 1] // 8)
    cx2[:R] = -(-b[:, 2] // 8)
    cy2[:R] = -(-b[:, 3] // 8)

    # sort by cx1 so each 128-ROI tile's x-windows fit a narrow col region
    order = np.argsort(cx1, kind="stable")
    cx1, cx2, cy1, cy2 = cx1[order], cx2[order], cy1[order], cy2[order]

    # per-ktile aligned x region [S8, S8+XW)
    spans = []
    starts = []
    for kt in range(NKT):
        lo = cx1[kt * 128:(kt + 1) * 128]
        hi = cx2[kt * 128:(kt + 1) * 128]
        real = lo < 129
        if real.any():
            s = int(lo[real].min())
            e = int(min(128, hi[real].max()))
        else:
            s, e = 0, 1
        spans.append(max(1, e - s))
        starts.append(s)
    span_max = max(spans)
    XW = 32
    while XW < span_max:
        XW *= 2
    XW = min(XW, 128)
    S8 = tuple(min(max(0, s), 128 - XW) for s in starts)
    NX = NKT * XW

    def colseg(arr):
        return np.ascontiguousarray(arr.reshape(NKT, 128).T)

    x1c, x2c = colseg(cx1), colseg(cx2)   # [128, NKT]
    y1c, y2c = colseg(cy1), colseg(cy2)
    s8a = np.asarray(S8, np.float32)[None, :]
    coords = np.empty((128, 4 * NKT + XW + 128 + 128), np.float16)
    coords[:, 0 * NKT:1 * NKT] = x1c - s8a     # local x bounds
    coords[:, 1 * NKT:2 * NKT] = x2c - s8a
    coords[:, 2 * NKT:3 * NKT] = y1c
    coords[:, 3 * NKT:4 * NKT] = y2c
    coords[:, 4 * NKT:4 * NKT + XW] = np.arange(XW, dtype=np.float16)[None, :]
    coords[:, 4 * NKT + XW:4 * NKT + XW + 128] = np.arange(
        128, dtype=np.float16)[None, :]
    coords[:, 4 * NKT + XW + 128:] = np.eye(128, dtype=np.float16)
    y1r = np.repeat(y1c[:, :, None], 128, axis=2).reshape(
        128, -1).astype(np.float16)
    y2r = np.repeat(y2c[:, :, None], 128, axis=2).reshape(
        128, -1).astype(np.float16)

    labels_f = labels.astype(np.float32).reshape(1, C)

    in_maps = []
    for core in range(NCORES):
        refc = np.zeros((128, NKT, 3, VCP), np.float32)
        packb = np.zeros((128, (2 * VCP + 2 * NIP) * 128), np.float32)
        packb[:, :2 * VCP * 128] = 1.0
        for v in range(VCP):
            gi = core + NCORES * v
            if gi < nv:
                ch = int(vidx[gi])
                col = np.zeros((3, RP), np.float32)
                col[:, :R] = refine[:, :, base + ch]
                col = col[:, order]
                refc[:, :, :, v] = col.reshape(3, NKT, 128).transpose(2, 1, 0)
                packb[:, v * 128:(v + 1) * 128] = blob[ch]
                packb[:, (VCP + v) * 128:(VCP + v + 1) * 128] = blob[ch].T
        for v in range(NIP):
            gi = core + NCORES * v
            if gi < ni:
                ch = int(iidx[gi])
                o = (2 * VCP + v) * 128
                packb[:, o:o + 128] = blob[ch]
                o = (2 * VCP + NIP + v) * 128
                packb[:, o:o + 128] = blob[ch].T
        in_maps.append({
            "refine": np.ascontiguousarray(
                refc.reshape(128, -1)).astype(np.float16),
            "coords": coords,
            "y1r": y1r,
            "y2r": y2r,
            "labels": labels_f,
            "packb": packb,
        })
    key = (VCP, NIP, NKT, C, XW, S8)
    return key, in_maps


def kernel(mil_result, refine_result, blob_conv, rois, labels, H, W,
           _trace=False):
    from concourse.bass_utils import run_bass_kernel_spmd

    key, in_maps = make_in_maps(mil_result, refine_result, blob_conv, rois,
                                labels, H, W)
    nc = _get_program(key)
    res = run_bass_kernel_spmd(nc, in_maps, core_ids=list(range(NCORES)),
                               trace=_trace)
    total = np.float64(0.0)
    for r in res.results:
        total += np.float64(r["out"][0, 0])
    out = np.array(total, dtype=np.float32)
    if _trace:
        kernel.last_results = res
    return out
